# revision 1
# baseline (speedup 1.0000x reference)
"""ChannelDropOut Trainium2 kernel.

Reference semantics (B,C,H,W = 64,512,32,32):
    gi        = X.mean(axis=(0,2,3))                      (C,)
    reward    = where(gi >= 0.5, 1.0, 0.0)
    new_alpha = alpha_param + reward
    new_beta  = beta_param + (1.0 - reward)
    keep_prob = Beta(alpha_param, beta_param)  sampled with jax key 42
    mask      = Bernoulli(keep_prob)                      (C,)
    out       = X * mask[None, :, None, None]

Device split: pure data parallel over batch across 8 NeuronCores. The
Beta/Bernoulli sampling is a tiny (C,)-sized op done host-side with jax CPU
(bit-identical to the reference, which cannot run on-neuron anyway since
jax.random.beta lowers to a while loop). The heavy work — one full read of X
for the per-channel sums plus the masked copy — runs on the cores.

The channel mask is known before the Bass program is built, so the program is
specialized to it: X is streamed through SBUF once (channels on partitions),
reduced per channel, and only *kept* channel runs are DMA'd back out. Dropped
channels are never written — run_bass_kernel_spmd zero-initializes output
buffers on both the native and the axon/PJRT path, so their zeros are free.
"""

import numpy as np

NCORES = 8
PT = 128  # SBUF partitions
REWARD_VALUE = 1.0

_prog_cache: dict = {}

# test.py hooks: set TRACE=True before calling kernel() to profile; the
# resulting BassKernelResults lands in LAST_RESULTS.
TRACE = False
LAST_RESULTS = None


def _compute_mask(alpha, beta):
    """Replicate the reference's sampling exactly, on jax CPU."""
    import jax

    cpu = jax.devices("cpu")[0]
    with jax.default_device(cpu):
        k_beta, k_bern = jax.random.split(jax.random.key(42))
        keep_prob = jax.random.beta(
            k_beta, jax.numpy.asarray(alpha), jax.numpy.asarray(beta)
        )
        mask = jax.random.bernoulli(k_bern, keep_prob)
        return np.asarray(mask)  # bool (C,)


def _runs_per_tile(mask, ct):
    """Maximal runs of kept channels, split at 128-channel tile boundaries.

    Returns [tile][(lo, hi)] with lo/hi relative to the tile's first channel.
    """
    per_tile = [[] for _ in range(ct)]
    c, C = 0, mask.shape[0]
    while c < C:
        if mask[c]:
            c0 = c
            while c < C and mask[c]:
                c += 1
            t0, t1 = c0 // PT, (c - 1) // PT
            for t in range(t0, t1 + 1):
                lo = max(c0, t * PT) - t * PT
                hi = min(c, (t + 1) * PT) - t * PT
                per_tile[t].append((lo, hi))
        else:
            c += 1
    return per_tile


def _build_program(bc, C, S, per_tile_runs):
    import concourse.bacc as bacc
    import concourse.mybir as mybir
    from concourse import tile

    ct = C // PT
    f32 = mybir.dt.float32
    nc = bacc.Bacc("TRN2", target_bir_lowering=False, debug=False, num_devices=NCORES)
    x = nc.dram_tensor("x", [bc, C, S], f32, kind="ExternalInput")
    out = nc.dram_tensor("out", [bc, C, S], f32, kind="ExternalOutput")
    gsum = nc.dram_tensor("gsum", [PT, ct], f32, kind="ExternalOutput")

    with tile.TileContext(nc) as tc:
        with (
            tc.tile_pool(name="sbuf", bufs=2) as pool,
            tc.tile_pool(name="accp", bufs=1) as accp,
        ):
            acc = accp.tile([PT, ct], f32)
            for t in range(ct):
                tl = pool.tile([PT, bc, S], f32)
                src = x[:, t * PT : (t + 1) * PT, :].rearrange("b c s -> c b s")
                nc.sync.dma_start(tl[:], src)
                nc.vector.reduce_sum(
                    acc[:, t : t + 1], tl[:], axis=mybir.AxisListType.XY
                )
                for lo, hi in per_tile_runs[t]:
                    dst = out[:, t * PT + lo : t * PT + hi, :].rearrange(
                        "b c s -> c b s"
                    )
                    nc.sync.dma_start(dst, tl[lo:hi, :, :])
            nc.sync.dma_start(gsum[:, :], acc[:])
    nc.compile()
    return nc


def kernel(X, alpha_param, beta_param, current_epoch):
    from concourse.bass_utils import run_bass_kernel_spmd

    global LAST_RESULTS

    X = np.ascontiguousarray(np.asarray(X, dtype=np.float32))
    alpha = np.asarray(alpha_param, dtype=np.float32)
    beta = np.asarray(beta_param, dtype=np.float32)
    B, C, H, W = X.shape
    S = H * W
    bc = B // NCORES

    mask = _compute_mask(alpha, beta)

    key = (mask.tobytes(), X.shape)
    if key not in _prog_cache:
        _prog_cache[key] = _build_program(bc, C, S, _runs_per_tile(mask, C // PT))
    nc = _prog_cache[key]

    X3 = X.reshape(B, C, S)
    in_maps = [{"x": X3[i * bc : (i + 1) * bc]} for i in range(NCORES)]
    res = run_bass_kernel_spmd(nc, in_maps, core_ids=list(range(NCORES)), trace=TRACE)
    LAST_RESULTS = res

    out = np.concatenate([r["out"] for r in res.results], axis=0).reshape(B, C, H, W)
    gsum = np.sum([r["gsum"] for r in res.results], axis=0, dtype=np.float32)
    gi = gsum.T.reshape(C) / np.float32(B * S)

    reward = np.where(gi >= 0.5, np.float32(REWARD_VALUE), np.float32(0.0))
    new_alpha = alpha + reward
    new_beta = beta + (np.float32(REWARD_VALUE) - reward)
    mask_proba = mask.astype(np.float32).reshape(1, C, 1, 1)
    return out, mask_proba, new_alpha, new_beta
